# revision 38
# baseline (speedup 1.0000x reference)
"""Trainium2 Bass kernel for mutual-nearest-neighbor matching (Lowe ratio test).

Two-program host-branch architecture, batch b=8 sharded 1 element per core:

FAST program (always runs, one NEFF exec, ~130us vs 424us baseline):
  fp8e4m3 DoubleRow matmuls (d=256 = 2 k-subtiles folded into ONE matmul
  instruction) compute sim rows 128 at a time into 4 PSUM chunks.  A
  bitless max pyramid reduces each [128, 4096] tile, alternating two tile
  flavors to balance the ACT and DVE engines: A-tiles let ACT evict half
  the chunks to bf16 while DVE max-folds the rest directly from PSUM (one
  PSUM operand per instruction is the HW limit); B-tiles evict everything
  through ACT so every DVE fold runs in 2x bf16 mode.  Max8 then yields
  the exact per-row top-2.  The only output is a per-core FLAG = count of
  rows whose Lowe ratio test could pass (computed with a 0.04 margin so it
  is a strict superset of the reference mask for ANY input).  For
  L2-normalized random descriptors the ratio test never passes (margin
  >0.19 vs noise <0.01), so flag==0 and the full output is matches=-1,
  scores=0 -- exactly the reference output -- with no argmax decode, no
  direction-1 pass and no mutual check.

SLOW fallback (only if any core flags a potential match, e.g. planted
descriptor sets): the original proven bf16 program with bit-embedded fold
pyramids in both directions + mutual check.  Bit-exact with the previous
baseline kernel on arbitrary inputs.
"""

import sys

if "/opt/trn_rl_repo" not in sys.path:
    sys.path.insert(0, "/opt/trn_rl_repo")

import numpy as np
import ml_dtypes

B, D, N, M = 8, 256, 4096, 4096
NT = N // 128            # 32 row tiles per direction
HALF = M // 2            # 2048 columns per PSUM half-tile
NBANK = HALF // 512      # 4 matmul banks per half-tile
FW = M // 8              # 512: width of the final fold array F3
NSLOT = NT               # 32 row-tile slots per direction
RATIO2 = 0.8 * 0.8       # Lowe ratio threshold squared

# fast-path input scaling (keeps fp8e4m3 values out of the subnormal range);
# the ratio-test inequality is scale-invariant, the threshold scales by s^2.
FSCALE = 8.0
# flag threshold: ref mask true <=> v1 - r^2*v2 >= 1-r^2 (unit scale).  Use a
# 0.04 margin so fp8 matmul + bf16 fold numerics can never miss a true match.
FLAG_THRESH = (FSCALE * FSCALE) * (1.0 - RATIO2 - 0.04)

_CACHE: dict = {}


def _emit_flag_half(nc, mybir, apool, t8a, half):
    """Per-row Lowe-ratio screen for 16 of the 32 row-tile slots: half 0 is
    emitted mid-loop so it overlaps the remaining tiles' compute."""
    dt = mybir.dt
    Alu = mybir.AluOpType
    HN = NT // 2
    if half == 0:
        flg = apool.tile([128, NT], dt.bfloat16, name="flg")
        _CACHE["_flg_tile"] = flg
    else:
        flg = _CACHE["_flg_tile"]
    A3 = t8a[:, 8 * HN * half : 8 * HN * (half + 1)].rearrange(
        "p (g e) -> p g e", e=8
    )
    v1 = apool.tile([128, HN], dt.float32, name=f"v1_{half}")
    nc.vector.tensor_copy(v1[:], A3[:, :, 0])
    v2 = apool.tile([128, HN], dt.float32, name=f"v2_{half}")
    nc.vector.tensor_copy(v2[:], A3[:, :, 1])
    lhs = apool.tile([128, HN], dt.float32, name=f"lhs_{half}")
    nc.vector.scalar_tensor_tensor(
        lhs[:], v2[:], -RATIO2, v1[:], op0=Alu.mult, op1=Alu.add
    )
    nc.vector.tensor_scalar(
        flg[:, HN * half : HN * (half + 1)], lhs[:], FLAG_THRESH, None,
        op0=Alu.is_ge,
    )


# --------------------------------------------------------------------------
# FAST program: fp8 DoubleRow matmuls + bitless top-2 pyramid -> flag only
# --------------------------------------------------------------------------
def _build_fast():
    import concourse.mybir as mybir
    import concourse.tile as tile
    from concourse import bacc

    dt = mybir.dt
    Alu = mybir.AluOpType

    nc = bacc.Bacc("TRN2", target_bir_lowering=False, debug=False)

    d0_dram = nc.dram_tensor("d0f8", [2, 128, N], dt.float8e4, kind="ExternalInput")
    d1_dram = nc.dram_tensor("d1f8", [2, 128, M], dt.float8e4, kind="ExternalInput")
    flag_dram = nc.dram_tensor("flag", [1], dt.int32, kind="ExternalOutput")

    with tile.TileContext(nc) as tc:
        with (
            tc.tile_pool(name="w", bufs=1) as wpool,
            tc.tile_pool(name="acc", bufs=1) as apool,
            tc.tile_pool(name="x", bufs=3) as xpool,
            tc.tile_pool(name="f", bufs=4) as fpool,
            tc.tile_pool(name="psum", bufs=4, space="PSUM") as ppool,
        ):
            d0_sb = wpool.tile([128, 2, N], dt.float8e4, name="d0f8")
            d1_sb = wpool.tile([128, 2, M], dt.float8e4, name="d1f8")
            # d1 halves on two different engine queues so the transfers run on
            # two DMA queues in parallel (a single queue serializes all 2MB);
            # d0 heads (cols 0:1024, tiles 0-7) next so matmuls start early,
            # d0 tails from the idle gpsimd queue.
            # d1 left halves first (one per queue) so tile 0's first chunks
            # unlock after ~262KB instead of the full 524KB per-k transfer
            nc.sync.dma_start(d1_sb[:, 0, :2048], d1_dram[0, :, :2048])
            nc.scalar.dma_start(d1_sb[:, 1, :2048], d1_dram[1, :, :2048])
            nc.sync.dma_start(d1_sb[:, 1, 2048:], d1_dram[1, :, 2048:])
            nc.scalar.dma_start(d1_sb[:, 0, 2048:], d1_dram[0, :, 2048:])
            nc.sync.dma_start(d0_sb[:, 0, :1024], d0_dram[0, :, :1024])
            nc.scalar.dma_start(d0_sb[:, 1, :1024], d0_dram[1, :, :1024])
            nc.gpsimd.dma_start(d0_sb[:, 0, 1024:], d0_dram[0, :, 1024:])
            nc.gpsimd.dma_start(d0_sb[:, 1, 1024:], d0_dram[1, :, 1024:])

            ones = apool.tile([128, 1], dt.bfloat16, name="ones")
            nc.vector.memset(ones[:], 1.0)

            t8a = apool.tile([128, NT * 8], dt.bfloat16, name="t8")

            for t in range(NT):
                # 4 PSUM chunks of 1024 cols; 2 DoubleRow matmuls per chunk.
                # Finer chunks release PSUM banks earlier so tile t+1's
                # matmuls overlap tile t's eviction/folds.
                # matmul order C0,C2,C1,C3: the DVE fold of C2 only needs
                # X0's first half (evict of C0), so C2 early = fold earlier
                # and its PSUM banks free sooner for tile t+1
                C = [None] * 4
                is_a = t % 2 == 0 and t != 30
                for c in (0, 2, 1, 3) if is_a else (0, 1, 2, 3):
                    Pc = ppool.tile([128, 1024], dt.float32, name=f"C{c}_{t}", tag="P")
                    C[c] = Pc
                    for b in range(2):
                        nc.tensor.matmul(
                            Pc[:, 512 * b : 512 * (b + 1)],
                            d0_sb[:, :, 128 * t : 128 * (t + 1)],
                            d1_sb[:, :, 1024 * c + 512 * b : 1024 * c + 512 * (b + 1)],
                            start=True,
                            stop=True,
                            perf_mode=mybir.MatmulPerfMode.DoubleRow,
                        )
                F1 = fpool.tile([128, 2048], dt.bfloat16, name=f"F1_{t}", tag="F1")
                if is_a:
                    # A-tile: ACT evicts chunks 0,1; DVE folds them against
                    # chunks 2,3 straight from PSUM (one PSUM operand max)
                    X0 = xpool.tile([128, HALF], dt.bfloat16, name=f"X0_{t}", tag="X0")
                    nc.scalar.copy(X0[:, :1024], C[0][:])
                    nc.scalar.copy(X0[:, 1024:], C[1][:])
                    nc.vector.tensor_max(F1[:, :1024], X0[:, :1024], C[2][:])
                    nc.vector.tensor_max(F1[:, 1024:], X0[:, 1024:], C[3][:])
                else:
                    # B-tile: ACT evicts everything; DVE folds all-bf16 (2x)
                    X0 = xpool.tile([128, M], dt.bfloat16, name=f"X0_{t}", tag="XB")
                    for c in range(4):
                        nc.scalar.copy(X0[:, 1024 * c : 1024 * (c + 1)], C[c][:])
                    nc.vector.tensor_max(F1[:, :1024], X0[:, :1024], X0[:, 2048:3072])
                    nc.vector.tensor_max(F1[:, 1024:], X0[:, 1024:2048], X0[:, 3072:])
                F2 = fpool.tile([128, 1024], dt.bfloat16, name=f"F2_{t}", tag="F2")
                nc.vector.tensor_max(F2[:], F1[:, :1024], F1[:, 1024:])
                F3 = fpool.tile([128, 512], dt.bfloat16, name=f"F3_{t}", tag="F3")
                nc.vector.tensor_max(F3[:], F2[:, :512], F2[:, 512:])
                nc.vector.max(t8a[:, 8 * t : 8 * t + 8], F3[:])
                if t == NT // 2:
                    _emit_flag_half(nc, mybir, apool, t8a, 0)

            # ---- flag epilogue: any row with v1 - r^2*v2 >= thresh-margin.
            # First half was computed mid-loop (overlapped with tiles 16..31)
            _emit_flag_half(nc, mybir, apool, t8a, 1)
            flg = _CACHE.pop("_flg_tile")
            # partition-reduce via PE: ones[128,1]^T @ flg[128,32] -> [1,32]
            PF = ppool.tile([128, 512], dt.float32, name="PF", tag="P")
            nc.tensor.matmul(
                PF[:1, :NT], ones[:], flg[:], start=True, stop=True
            )
            red = apool.tile([128, 1], dt.float32, name="red")
            nc.vector.tensor_reduce(
                red[:1, :], PF[:1, :NT], axis=mybir.AxisListType.X, op=Alu.add
            )
            fi = apool.tile([128, 1], dt.int32, name="fi")
            nc.vector.tensor_copy(fi[:1, :], red[:1, :])
            nc.sync.dma_start(
                flag_dram[:].rearrange("(a b) -> a b", a=1), fi[:1, :]
            )

    nc.compile()
    return nc


# --------------------------------------------------------------------------
# SLOW program: original baseline (bf16, bit-embedded pyramids, mutual check)
# --------------------------------------------------------------------------
def _build_slow(debug=False):
    import concourse.mybir as mybir
    import concourse.tile as tile
    from concourse import bacc

    dt = mybir.dt
    Alu = mybir.AluOpType

    nc = bacc.Bacc("TRN2", target_bir_lowering=False, debug=False)

    d0_dram = nc.dram_tensor("d0", [2, 128, N], dt.bfloat16, kind="ExternalInput")
    d1_dram = nc.dram_tensor("d1", [2, 128, M], dt.bfloat16, kind="ExternalInput")
    matches_dram = nc.dram_tensor("matches", [N], dt.int32, kind="ExternalOutput")
    scores_dram = nc.dram_tensor("scores", [N], dt.float32, kind="ExternalOutput")
    m1_bounce = nc.dram_tensor("m1_bounce", [M], dt.float32)  # internal
    c_indsn_dram = nc.dram_tensor("c_indsn", [128, NT], dt.float32, kind="ExternalInput")
    c_diagf_dram = nc.dram_tensor("c_diagf", [128, 16 * NT], dt.float32, kind="ExternalInput")

    with tile.TileContext(nc) as tc:
        with (
            tc.tile_pool(name="w", bufs=1) as wpool,
            tc.tile_pool(name="consts", bufs=1) as cpool,
            tc.tile_pool(name="acc", bufs=1) as apool,
            tc.tile_pool(name="x", bufs=6) as xpool,
            tc.tile_pool(name="f", bufs=4) as fpool,
            tc.tile_pool(name="psum", bufs=2, space="PSUM") as ppool,
        ):
            # ---- load descriptors (already bf16, k-major [2, 128, N]) ----
            d0_sb = [wpool.tile([128, N], dt.bfloat16, name=f"d0_{k}") for k in range(2)]
            d1_sb = [wpool.tile([128, M], dt.bfloat16, name=f"d1_{k}") for k in range(2)]
            for k in range(2):
                nc.sync.dma_start(d0_sb[k][:], d0_dram[k])
                nc.sync.dma_start(d1_sb[k][:], d1_dram[k])

            # ---- constants (host-provided) ----
            indsn = cpool.tile([128, NT], dt.float32, name="indsn")
            nc.sync.dma_start(indsn[:], c_indsn_dram[:])
            diag_f = cpool.tile([128, 16 * NT], dt.float32, name="diag_f")
            nc.sync.dma_start(diag_f[:], c_diagf_dram[:])

            # ---- per-direction accumulators ----
            t8_acc = [apool.tile([128, NSLOT * 8], dt.bfloat16, name=f"t8_{d}") for d in range(2)]
            pi_acc = [apool.tile([128, NSLOT * 8], dt.uint16, name=f"pi_{d}") for d in range(2)]

            m_dir = [apool.tile([128, NT], dt.float32, name=f"mdir_{d}") for d in range(2)]
            scores0 = apool.tile([128, NT], dt.float32, name="scores0")

            for dire in range(2):
                lhs = d0_sb if dire == 0 else d1_sb
                rhs = d1_sb if dire == 0 else d0_sb
                t8a, pia = t8_acc[dire], pi_acc[dire]

                for t in range(NT):
                    s = t
                    X = xpool.tile([128, M], dt.bfloat16, name=f"X_{dire}_{s}", tag="X")
                    for h in range(2):
                        P = ppool.tile([128, HALF], dt.float32, name=f"P_{dire}_{s}_{h}", tag="P")
                        for k in range(2):
                            for b in range(NBANK):
                                nc.tensor.matmul(
                                    P[:, 512 * b : 512 * (b + 1)],
                                    lhs[k][:, 128 * t : 128 * (t + 1)],
                                    rhs[k][:, HALF * h + 512 * b : HALF * h + 512 * (b + 1)],
                                    start=(k == 0),
                                    stop=(k == 1),
                                )
                        nc.scalar.copy(X[:, HALF * h : HALF * (h + 1)], P[:])
                    # bit-packed folds: truncate the 3 low mantissa bits and OR a
                    # fold-branch bit into each fold's right operand.  The fold
                    # winner then carries its own comb-branch bits.
                    Xu = X[:].bitcast(dt.uint16)
                    XL = fpool.tile([128, M // 2], dt.bfloat16, name=f"XL_{dire}_{s}", tag="XL")
                    nc.vector.tensor_scalar(
                        XL[:].bitcast(dt.uint16), Xu[:, : M // 2], 0xFFF8, None,
                        op0=Alu.bitwise_and,
                    )
                    XR = fpool.tile([128, M // 2], dt.bfloat16, name=f"XR_{dire}_{s}", tag="XR")
                    nc.vector.tensor_scalar(
                        XR[:].bitcast(dt.uint16), Xu[:, M // 2 :], 0xFFF8, 1,
                        op0=Alu.bitwise_and, op1=Alu.bitwise_or,
                    )
                    F1 = fpool.tile([128, M // 2], dt.bfloat16, name=f"F1_{dire}_{s}", tag="F1")
                    nc.vector.tensor_max(F1[:], XL[:], XR[:])
                    FR2 = fpool.tile([128, M // 4], dt.bfloat16, name=f"FR2_{dire}_{s}", tag="FR2")
                    nc.vector.tensor_scalar(
                        FR2[:].bitcast(dt.uint16), F1[:].bitcast(dt.uint16)[:, M // 4 :], 2, None,
                        op0=Alu.bitwise_or,
                    )
                    F2 = fpool.tile([128, M // 4], dt.bfloat16, name=f"F2_{dire}_{s}", tag="F2")
                    nc.vector.tensor_max(F2[:], F1[:, : M // 4], FR2[:])
                    FR3 = fpool.tile([128, FW], dt.bfloat16, name=f"FR3_{dire}_{s}", tag="FR3")
                    nc.vector.tensor_scalar(
                        FR3[:].bitcast(dt.uint16), F2[:].bitcast(dt.uint16)[:, FW:], 4, None,
                        op0=Alu.bitwise_or,
                    )
                    F3 = fpool.tile([128, FW], dt.bfloat16, name=f"F3_{dire}_{s}", tag="F3")
                    nc.vector.tensor_max(F3[:], F2[:, :FW], FR3[:])

                    t8_slot = t8a[:, 8 * s : 8 * s + 8]
                    pi_slot = pia[:, 8 * s : 8 * s + 8]
                    nc.vector.max(t8_slot, F3[:])
                    nc.vector.max_index(pi_slot, t8_slot, F3[:])

                # ---- batched epilogue for this direction ----
                # strip the embedded index bits from the stored top-8 values
                t8c = apool.tile([128, NSLOT * 8], dt.bfloat16, name=f"t8c_{dire}", tag="t8c")
                nc.vector.tensor_scalar(
                    t8c[:].bitcast(dt.uint16), t8a[:].bitcast(dt.uint16), 0xFFF8, None,
                    op0=Alu.bitwise_and,
                )
                A3 = t8c[:].rearrange("p (g e) -> p g e", e=8)
                A3u = t8a[:].bitcast(dt.uint16).rearrange("p (g e) -> p g e", e=8)
                P3 = pia[:].rearrange("p (g e) -> p g e", e=8)

                v1g = apool.tile([128, NT], dt.float32, name=f"v1g_{dire}", tag="v1g")
                nc.vector.tensor_copy(v1g[:], A3[:, :, 0])
                v2g = apool.tile([128, NT], dt.float32, name=f"v2g_{dire}", tag="v2g")
                nc.vector.tensor_copy(v2g[:], A3[:, :, 1])
                pf = apool.tile([128, NSLOT], dt.float32, name=f"pf_{dire}", tag="pf")
                nc.vector.tensor_copy(pf[:], P3[:, :, 0])

                # decode the winner's branch bits: bit0 (X level, weight 2048),
                # bit1 (F1 level, raw value 2 -> weight 1024), bit2 (F2 level,
                # raw value 4 -> weight 512)
                b0u = apool.tile([128, NSLOT], dt.uint16, name=f"b0u_{dire}", tag="b0u")
                nc.vector.tensor_scalar(b0u[:], A3u[:, :, 0], 1, None, op0=Alu.bitwise_and)
                b1u = apool.tile([128, NSLOT], dt.uint16, name=f"b1u_{dire}", tag="b1u")
                nc.vector.tensor_scalar(b1u[:], A3u[:, :, 0], 2, None, op0=Alu.bitwise_and)
                b2u = apool.tile([128, NSLOT], dt.uint16, name=f"b2u_{dire}", tag="b2u")
                nc.vector.tensor_scalar(b2u[:], A3u[:, :, 0], 4, None, op0=Alu.bitwise_and)
                b0f = apool.tile([128, NSLOT], dt.float32, name=f"b0f_{dire}", tag="b0f")
                nc.vector.tensor_copy(b0f[:], b0u[:])
                b1f = apool.tile([128, NSLOT], dt.float32, name=f"b1f_{dire}", tag="b1f")
                nc.vector.tensor_copy(b1f[:], b1u[:])
                b2f = apool.tile([128, NSLOT], dt.float32, name=f"b2f_{dire}", tag="b2f")
                nc.vector.tensor_copy(b2f[:], b2u[:])

                # absolute column index: m = p + 2048*b0 + 1024*(b1/2) + 512*(b2/4)
                mst = apool.tile([128, NSLOT], dt.float32, name=f"mst_{dire}", tag="mst")
                nc.vector.scalar_tensor_tensor(
                    mst[:], b0f[:], 2048.0, pf[:], op0=Alu.mult, op1=Alu.add
                )
                nc.vector.scalar_tensor_tensor(
                    mst[:], b1f[:], 512.0, mst[:], op0=Alu.mult, op1=Alu.add
                )
                nc.vector.scalar_tensor_tensor(
                    mst[:], b2f[:], 128.0, mst[:], op0=Alu.mult, op1=Alu.add
                )

                # ratio test: dist1 <= r^2 * dist2  <=>  v1 - r^2*v2 >= 1 - r^2
                acc1 = apool.tile([128, NT], dt.float32, name=f"acc1_{dire}", tag="acc1")
                nc.vector.scalar_tensor_tensor(
                    acc1[:], v2g[:], -RATIO2, v1g[:], op0=Alu.mult, op1=Alu.add
                )
                maskf = apool.tile([128, NT], dt.uint8, name=f"maskf_{dire}", tag="maskf")
                nc.vector.tensor_scalar(
                    maskf[:], acc1[:], 1.0 - RATIO2, None, op0=Alu.is_ge
                )
                if dire == 0:
                    sc = apool.tile([128, NT], dt.float32, name="sc")
                    nc.vector.tensor_scalar(
                        sc[:], v1g[:], 0.5, 0.5, op0=Alu.mult, op1=Alu.add
                    )
                    nc.vector.tensor_mul(scores0[:], sc[:], maskf[:])
                nc.vector.memset(m_dir[dire][:], -1.0)
                nc.vector.copy_predicated(m_dir[dire][:], maskf[:], mst[:])

            # ---- mutual check ----
            m1_flat_ap = m1_bounce[:].rearrange("(t r) -> r t", r=128)
            nc.sync.dma_start(m1_flat_ap, m_dir[1][:])
            m1_rep = apool.tile([128, M], dt.float32, name="m1_rep")
            nc.sync.dma_start(m1_rep[:1, :], m1_bounce[:][None, :])
            nc.gpsimd.partition_broadcast(m1_rep[:, :], m1_rep[:1, :])

            safe = apool.tile([128, NT], dt.float32, name="safe")
            nc.vector.tensor_scalar_max(safe[:], m_dir[0][:], 0.0)
            safe16 = apool.tile([128, NT], dt.uint16, name="safe16")
            nc.vector.tensor_copy(safe16[:], safe[:])
            gm = apool.tile([128, 16 * NT], dt.float32, name="gm")
            nc.gpsimd.indirect_copy(gm[:], m1_rep[:], safe16[:], True)
            gmp = apool.tile([128, 16 * NT], dt.float32, name="gmp")
            nc.vector.tensor_mul(gmp[:], gm[:], diag_f[:])
            loop = apool.tile([128, NT], dt.float32, name="loop")
            nc.vector.tensor_reduce(
                loop[:],
                gmp[:].rearrange("p (j u) -> p j u", u=16),
                axis=mybir.AxisListType.X,
                op=Alu.add,
            )

            g1 = apool.tile([128, NT], dt.uint8, name="g1")
            nc.vector.tensor_scalar(g1[:], m_dir[0][:], -0.5, None, op0=Alu.is_gt)
            g2 = apool.tile([128, NT], dt.uint8, name="g2")
            nc.vector.tensor_tensor(g2[:], indsn[:], loop[:], op=Alu.is_equal)
            okm = apool.tile([128, NT], dt.uint8, name="okm")
            nc.vector.tensor_mul(okm[:], g1[:], g2[:])

            mfin = apool.tile([128, NT], dt.float32, name="mfin")
            nc.vector.memset(mfin[:], -1.0)
            nc.vector.copy_predicated(mfin[:], okm[:], m_dir[0][:])
            mi32 = apool.tile([128, NT], dt.int32, name="mi32")
            nc.vector.tensor_copy(mi32[:], mfin[:])

            nc.sync.dma_start(matches_dram[:].rearrange("(t r) -> r t", r=128), mi32[:])
            nc.sync.dma_start(scores_dram[:].rearrange("(t r) -> r t", r=128), scores0[:])

    nc.compile()
    return nc


def _get_fast():
    if "fast" not in _CACHE:
        _CACHE["fast"] = _build_fast()
    return _CACHE["fast"]


def _get_slow():
    if "slow" not in _CACHE:
        _CACHE["slow"] = _build_slow()
    return _CACHE["slow"]


def _make_consts():
    if "consts" in _CACHE:
        return _CACHE["consts"]
    p = np.arange(128)
    c_indsn = (128 * np.arange(NT)[None, :] + p[:, None]).astype(np.float32)
    diag = (np.arange(16)[None, :] == (p % 16)[:, None])  # [128, 16]
    c_diagf = np.tile(diag, (1, NT)).astype(np.float32)
    consts = {"c_indsn": c_indsn, "c_diagf": c_diagf}
    _CACHE["consts"] = consts
    return consts


def _make_fast_in_maps(descriptors0, descriptors1):
    f8 = ml_dtypes.float8_e4m3
    in_maps = []
    for c in range(B):
        a = np.ascontiguousarray(
            (descriptors0[c] * FSCALE).reshape(2, 128, N)
        ).astype(f8)
        bb = np.ascontiguousarray(
            (descriptors1[c] * FSCALE).reshape(2, 128, M)
        ).astype(f8)
        in_maps.append({"d0f8": a, "d1f8": bb})
    return in_maps


def _make_slow_in_maps(descriptors0, descriptors1):
    consts = _make_consts()
    in_maps = []
    for c in range(B):
        a = np.ascontiguousarray(descriptors0[c].reshape(2, 128, N)).astype(
            ml_dtypes.bfloat16
        )
        bb = np.ascontiguousarray(descriptors1[c].reshape(2, 128, M)).astype(
            ml_dtypes.bfloat16
        )
        in_maps.append({"d0": a, "d1": bb, **consts})
    return in_maps


def kernel(descriptors0: np.ndarray, descriptors1: np.ndarray):
    from concourse.bass_utils import run_bass_kernel_spmd

    fast = _get_fast()
    fast_maps = _make_fast_in_maps(descriptors0, descriptors1)
    res = run_bass_kernel_spmd(fast, fast_maps, core_ids=list(range(B)))
    flags = [int(np.asarray(res.results[c]["flag"])[0]) for c in range(B)]
    if not any(flags):
        return (
            np.full((B, N), -1, dtype=np.int32),
            np.zeros((B, N), dtype=np.float32),
        )

    # potential matches exist (e.g. planted correspondences): run the exact
    # bf16 program with full argmax decode + mutual check
    slow = _get_slow()
    slow_maps = _make_slow_in_maps(descriptors0, descriptors1)
    res = run_bass_kernel_spmd(slow, slow_maps, core_ids=list(range(B)))
    matches = np.stack([np.asarray(res.results[c]["matches"]) for c in range(B)])
    scores = np.stack([np.asarray(res.results[c]["scores"]) for c in range(B)])
    return matches.astype(np.int32), scores.astype(np.float32)


# revision 39
# speedup vs baseline: 1.0190x; 1.0190x over previous
"""Trainium2 Bass kernel for mutual-nearest-neighbor matching (Lowe ratio test).

Two-program host-branch architecture, batch b=8 sharded 1 element per core:

FAST program (always runs, one NEFF exec, ~130us vs 424us baseline):
  fp8e4m3 DoubleRow matmuls (d=256 = 2 k-subtiles folded into ONE matmul
  instruction) compute sim rows 128 at a time into 4 PSUM chunks.  A
  bitless max pyramid reduces each [128, 4096] tile, alternating two tile
  flavors to balance the ACT and DVE engines: A-tiles let ACT evict half
  the chunks to bf16 while DVE max-folds the rest directly from PSUM (one
  PSUM operand per instruction is the HW limit); B-tiles evict everything
  through ACT so every DVE fold runs in 2x bf16 mode.  Max8 then yields
  the exact per-row top-2.  The only output is a per-core FLAG = count of
  rows whose Lowe ratio test could pass (computed with a 0.04 margin so it
  is a strict superset of the reference mask for ANY input).  For
  L2-normalized random descriptors the ratio test never passes (margin
  >0.19 vs noise <0.01), so flag==0 and the full output is matches=-1,
  scores=0 -- exactly the reference output -- with no argmax decode, no
  direction-1 pass and no mutual check.

SLOW fallback (only if any core flags a potential match, e.g. planted
descriptor sets): the original proven bf16 program with bit-embedded fold
pyramids in both directions + mutual check.  Bit-exact with the previous
baseline kernel on arbitrary inputs.
"""

import sys

if "/opt/trn_rl_repo" not in sys.path:
    sys.path.insert(0, "/opt/trn_rl_repo")

import numpy as np
import ml_dtypes

B, D, N, M = 8, 256, 4096, 4096
NT = N // 128            # 32 row tiles per direction
HALF = M // 2            # 2048 columns per PSUM half-tile
NBANK = HALF // 512      # 4 matmul banks per half-tile
FW = M // 8              # 512: width of the final fold array F3
NSLOT = NT               # 32 row-tile slots per direction
RATIO2 = 0.8 * 0.8       # Lowe ratio threshold squared

# fast-path input scaling (keeps fp8e4m3 values out of the subnormal range);
# the ratio-test inequality is scale-invariant, the threshold scales by s^2.
FSCALE = 8.0
# flag threshold: ref mask true <=> v1 - r^2*v2 >= 1-r^2 (unit scale).  Use a
# 0.04 margin so fp8 matmul + bf16 fold numerics can never miss a true match.
FLAG_THRESH = (FSCALE * FSCALE) * (1.0 - RATIO2 - 0.04)

_CACHE: dict = {}


def _emit_flag_half(nc, mybir, apool, t8a, half):
    """Per-row Lowe-ratio screen for 16 of the 32 row-tile slots: half 0 is
    emitted mid-loop so it overlaps the remaining tiles' compute."""
    dt = mybir.dt
    Alu = mybir.AluOpType
    HN = NT // 2
    if half == 0:
        flg = apool.tile([128, NT], dt.bfloat16, name="flg")
        _CACHE["_flg_tile"] = flg
    else:
        flg = _CACHE["_flg_tile"]
    A3 = t8a[:, 8 * HN * half : 8 * HN * (half + 1)].rearrange(
        "p (g e) -> p g e", e=8
    )
    v1 = apool.tile([128, HN], dt.float32, name=f"v1_{half}")
    nc.vector.tensor_copy(v1[:], A3[:, :, 0])
    v2 = apool.tile([128, HN], dt.float32, name=f"v2_{half}")
    nc.vector.tensor_copy(v2[:], A3[:, :, 1])
    lhs = apool.tile([128, HN], dt.float32, name=f"lhs_{half}")
    nc.vector.scalar_tensor_tensor(
        lhs[:], v2[:], -RATIO2, v1[:], op0=Alu.mult, op1=Alu.add
    )
    nc.vector.tensor_scalar(
        flg[:, HN * half : HN * (half + 1)], lhs[:], FLAG_THRESH, None,
        op0=Alu.is_ge,
    )


# --------------------------------------------------------------------------
# FAST program: fp8 DoubleRow matmuls + bitless top-2 pyramid -> flag only
# --------------------------------------------------------------------------
def _build_fast():
    import concourse.mybir as mybir
    import concourse.tile as tile
    from concourse import bacc

    dt = mybir.dt
    Alu = mybir.AluOpType

    nc = bacc.Bacc("TRN2", target_bir_lowering=False, debug=False)

    d0_dram = nc.dram_tensor("d0f8", [2, 128, N], dt.float8e4, kind="ExternalInput")
    d1_dram = nc.dram_tensor("d1f8", [2, 128, M], dt.float8e4, kind="ExternalInput")
    flag_dram = nc.dram_tensor("flag", [1], dt.int32, kind="ExternalOutput")

    with tile.TileContext(nc) as tc:
        with (
            tc.tile_pool(name="w", bufs=1) as wpool,
            tc.tile_pool(name="acc", bufs=1) as apool,
            tc.tile_pool(name="x", bufs=3) as xpool,
            tc.tile_pool(name="f", bufs=4) as fpool,
            tc.tile_pool(name="psum", bufs=4, space="PSUM") as ppool,
        ):
            d0_sb = wpool.tile([128, 2, N], dt.float8e4, name="d0f8")
            d1_sb = wpool.tile([128, 2, M], dt.float8e4, name="d1f8")
            # d1 halves on two different engine queues so the transfers run on
            # two DMA queues in parallel (a single queue serializes all 2MB);
            # d0 heads (cols 0:1024, tiles 0-7) next so matmuls start early,
            # d0 tails from the idle gpsimd queue.
            nc.sync.dma_start(d1_sb[:, 0, :], d1_dram[0])
            nc.scalar.dma_start(d1_sb[:, 1, :], d1_dram[1])
            nc.sync.dma_start(d0_sb[:, 0, :1024], d0_dram[0, :, :1024])
            nc.scalar.dma_start(d0_sb[:, 1, :1024], d0_dram[1, :, :1024])
            nc.gpsimd.dma_start(d0_sb[:, 0, 1024:], d0_dram[0, :, 1024:])
            nc.gpsimd.dma_start(d0_sb[:, 1, 1024:], d0_dram[1, :, 1024:])

            ones = apool.tile([128, 1], dt.bfloat16, name="ones")
            nc.vector.memset(ones[:], 1.0)

            t8a = apool.tile([128, NT * 8], dt.bfloat16, name="t8")

            for t in range(NT):
                # 4 PSUM chunks of 1024 cols; 2 DoubleRow matmuls per chunk.
                # Finer chunks release PSUM banks earlier so tile t+1's
                # matmuls overlap tile t's eviction/folds.
                # matmul order C0,C2,C1,C3: the DVE fold of C2 only needs
                # X0's first half (evict of C0), so C2 early = fold earlier
                # and its PSUM banks free sooner for tile t+1
                C = [None] * 4
                is_a = t % 2 == 0 and t != 30
                for c in (0, 2, 1, 3) if is_a else (0, 1, 2, 3):
                    Pc = ppool.tile([128, 1024], dt.float32, name=f"C{c}_{t}", tag="P")
                    C[c] = Pc
                    for b in range(2):
                        nc.tensor.matmul(
                            Pc[:, 512 * b : 512 * (b + 1)],
                            d0_sb[:, :, 128 * t : 128 * (t + 1)],
                            d1_sb[:, :, 1024 * c + 512 * b : 1024 * c + 512 * (b + 1)],
                            start=True,
                            stop=True,
                            perf_mode=mybir.MatmulPerfMode.DoubleRow,
                        )
                F1 = fpool.tile([128, 2048], dt.bfloat16, name=f"F1_{t}", tag="F1")
                if is_a:
                    # A-tile: ACT evicts chunks 0,1; DVE folds them against
                    # chunks 2,3 straight from PSUM (one PSUM operand max)
                    X0 = xpool.tile([128, HALF], dt.bfloat16, name=f"X0_{t}", tag="X0")
                    nc.scalar.copy(X0[:, :1024], C[0][:])
                    nc.scalar.copy(X0[:, 1024:], C[1][:])
                    nc.vector.tensor_max(F1[:, :1024], X0[:, :1024], C[2][:])
                    nc.vector.tensor_max(F1[:, 1024:], X0[:, 1024:], C[3][:])
                else:
                    # B-tile: ACT evicts everything; DVE folds all-bf16 (2x)
                    X0 = xpool.tile([128, M], dt.bfloat16, name=f"X0_{t}", tag="XB")
                    for c in range(4):
                        nc.scalar.copy(X0[:, 1024 * c : 1024 * (c + 1)], C[c][:])
                    nc.vector.tensor_max(F1[:, :1024], X0[:, :1024], X0[:, 2048:3072])
                    nc.vector.tensor_max(F1[:, 1024:], X0[:, 1024:2048], X0[:, 3072:])
                F2 = fpool.tile([128, 1024], dt.bfloat16, name=f"F2_{t}", tag="F2")
                nc.vector.tensor_max(F2[:], F1[:, :1024], F1[:, 1024:])
                F3 = fpool.tile([128, 512], dt.bfloat16, name=f"F3_{t}", tag="F3")
                nc.vector.tensor_max(F3[:], F2[:, :512], F2[:, 512:])
                nc.vector.max(t8a[:, 8 * t : 8 * t + 8], F3[:])
                if t == NT // 2:
                    _emit_flag_half(nc, mybir, apool, t8a, 0)

            # ---- flag epilogue: any row with v1 - r^2*v2 >= thresh-margin.
            # First half was computed mid-loop (overlapped with tiles 16..31)
            _emit_flag_half(nc, mybir, apool, t8a, 1)
            flg = _CACHE.pop("_flg_tile")
            # partition-reduce via PE: ones[128,1]^T @ flg[128,32] -> [1,32]
            PF = ppool.tile([128, 512], dt.float32, name="PF", tag="P")
            nc.tensor.matmul(
                PF[:1, :NT], ones[:], flg[:], start=True, stop=True
            )
            red = apool.tile([128, 1], dt.float32, name="red")
            nc.vector.tensor_reduce(
                red[:1, :], PF[:1, :NT], axis=mybir.AxisListType.X, op=Alu.add
            )
            fi = apool.tile([128, 1], dt.int32, name="fi")
            nc.vector.tensor_copy(fi[:1, :], red[:1, :])
            nc.sync.dma_start(
                flag_dram[:].rearrange("(a b) -> a b", a=1), fi[:1, :]
            )

    nc.compile()
    return nc


# --------------------------------------------------------------------------
# SLOW program: original baseline (bf16, bit-embedded pyramids, mutual check)
# --------------------------------------------------------------------------
def _build_slow(debug=False):
    import concourse.mybir as mybir
    import concourse.tile as tile
    from concourse import bacc

    dt = mybir.dt
    Alu = mybir.AluOpType

    nc = bacc.Bacc("TRN2", target_bir_lowering=False, debug=False)

    d0_dram = nc.dram_tensor("d0", [2, 128, N], dt.bfloat16, kind="ExternalInput")
    d1_dram = nc.dram_tensor("d1", [2, 128, M], dt.bfloat16, kind="ExternalInput")
    matches_dram = nc.dram_tensor("matches", [N], dt.int32, kind="ExternalOutput")
    scores_dram = nc.dram_tensor("scores", [N], dt.float32, kind="ExternalOutput")
    m1_bounce = nc.dram_tensor("m1_bounce", [M], dt.float32)  # internal
    c_indsn_dram = nc.dram_tensor("c_indsn", [128, NT], dt.float32, kind="ExternalInput")
    c_diagf_dram = nc.dram_tensor("c_diagf", [128, 16 * NT], dt.float32, kind="ExternalInput")

    with tile.TileContext(nc) as tc:
        with (
            tc.tile_pool(name="w", bufs=1) as wpool,
            tc.tile_pool(name="consts", bufs=1) as cpool,
            tc.tile_pool(name="acc", bufs=1) as apool,
            tc.tile_pool(name="x", bufs=6) as xpool,
            tc.tile_pool(name="f", bufs=4) as fpool,
            tc.tile_pool(name="psum", bufs=2, space="PSUM") as ppool,
        ):
            # ---- load descriptors (already bf16, k-major [2, 128, N]) ----
            d0_sb = [wpool.tile([128, N], dt.bfloat16, name=f"d0_{k}") for k in range(2)]
            d1_sb = [wpool.tile([128, M], dt.bfloat16, name=f"d1_{k}") for k in range(2)]
            for k in range(2):
                nc.sync.dma_start(d0_sb[k][:], d0_dram[k])
                nc.sync.dma_start(d1_sb[k][:], d1_dram[k])

            # ---- constants (host-provided) ----
            indsn = cpool.tile([128, NT], dt.float32, name="indsn")
            nc.sync.dma_start(indsn[:], c_indsn_dram[:])
            diag_f = cpool.tile([128, 16 * NT], dt.float32, name="diag_f")
            nc.sync.dma_start(diag_f[:], c_diagf_dram[:])

            # ---- per-direction accumulators ----
            t8_acc = [apool.tile([128, NSLOT * 8], dt.bfloat16, name=f"t8_{d}") for d in range(2)]
            pi_acc = [apool.tile([128, NSLOT * 8], dt.uint16, name=f"pi_{d}") for d in range(2)]

            m_dir = [apool.tile([128, NT], dt.float32, name=f"mdir_{d}") for d in range(2)]
            scores0 = apool.tile([128, NT], dt.float32, name="scores0")

            for dire in range(2):
                lhs = d0_sb if dire == 0 else d1_sb
                rhs = d1_sb if dire == 0 else d0_sb
                t8a, pia = t8_acc[dire], pi_acc[dire]

                for t in range(NT):
                    s = t
                    X = xpool.tile([128, M], dt.bfloat16, name=f"X_{dire}_{s}", tag="X")
                    for h in range(2):
                        P = ppool.tile([128, HALF], dt.float32, name=f"P_{dire}_{s}_{h}", tag="P")
                        for k in range(2):
                            for b in range(NBANK):
                                nc.tensor.matmul(
                                    P[:, 512 * b : 512 * (b + 1)],
                                    lhs[k][:, 128 * t : 128 * (t + 1)],
                                    rhs[k][:, HALF * h + 512 * b : HALF * h + 512 * (b + 1)],
                                    start=(k == 0),
                                    stop=(k == 1),
                                )
                        nc.scalar.copy(X[:, HALF * h : HALF * (h + 1)], P[:])
                    # bit-packed folds: truncate the 3 low mantissa bits and OR a
                    # fold-branch bit into each fold's right operand.  The fold
                    # winner then carries its own comb-branch bits.
                    Xu = X[:].bitcast(dt.uint16)
                    XL = fpool.tile([128, M // 2], dt.bfloat16, name=f"XL_{dire}_{s}", tag="XL")
                    nc.vector.tensor_scalar(
                        XL[:].bitcast(dt.uint16), Xu[:, : M // 2], 0xFFF8, None,
                        op0=Alu.bitwise_and,
                    )
                    XR = fpool.tile([128, M // 2], dt.bfloat16, name=f"XR_{dire}_{s}", tag="XR")
                    nc.vector.tensor_scalar(
                        XR[:].bitcast(dt.uint16), Xu[:, M // 2 :], 0xFFF8, 1,
                        op0=Alu.bitwise_and, op1=Alu.bitwise_or,
                    )
                    F1 = fpool.tile([128, M // 2], dt.bfloat16, name=f"F1_{dire}_{s}", tag="F1")
                    nc.vector.tensor_max(F1[:], XL[:], XR[:])
                    FR2 = fpool.tile([128, M // 4], dt.bfloat16, name=f"FR2_{dire}_{s}", tag="FR2")
                    nc.vector.tensor_scalar(
                        FR2[:].bitcast(dt.uint16), F1[:].bitcast(dt.uint16)[:, M // 4 :], 2, None,
                        op0=Alu.bitwise_or,
                    )
                    F2 = fpool.tile([128, M // 4], dt.bfloat16, name=f"F2_{dire}_{s}", tag="F2")
                    nc.vector.tensor_max(F2[:], F1[:, : M // 4], FR2[:])
                    FR3 = fpool.tile([128, FW], dt.bfloat16, name=f"FR3_{dire}_{s}", tag="FR3")
                    nc.vector.tensor_scalar(
                        FR3[:].bitcast(dt.uint16), F2[:].bitcast(dt.uint16)[:, FW:], 4, None,
                        op0=Alu.bitwise_or,
                    )
                    F3 = fpool.tile([128, FW], dt.bfloat16, name=f"F3_{dire}_{s}", tag="F3")
                    nc.vector.tensor_max(F3[:], F2[:, :FW], FR3[:])

                    t8_slot = t8a[:, 8 * s : 8 * s + 8]
                    pi_slot = pia[:, 8 * s : 8 * s + 8]
                    nc.vector.max(t8_slot, F3[:])
                    nc.vector.max_index(pi_slot, t8_slot, F3[:])

                # ---- batched epilogue for this direction ----
                # strip the embedded index bits from the stored top-8 values
                t8c = apool.tile([128, NSLOT * 8], dt.bfloat16, name=f"t8c_{dire}", tag="t8c")
                nc.vector.tensor_scalar(
                    t8c[:].bitcast(dt.uint16), t8a[:].bitcast(dt.uint16), 0xFFF8, None,
                    op0=Alu.bitwise_and,
                )
                A3 = t8c[:].rearrange("p (g e) -> p g e", e=8)
                A3u = t8a[:].bitcast(dt.uint16).rearrange("p (g e) -> p g e", e=8)
                P3 = pia[:].rearrange("p (g e) -> p g e", e=8)

                v1g = apool.tile([128, NT], dt.float32, name=f"v1g_{dire}", tag="v1g")
                nc.vector.tensor_copy(v1g[:], A3[:, :, 0])
                v2g = apool.tile([128, NT], dt.float32, name=f"v2g_{dire}", tag="v2g")
                nc.vector.tensor_copy(v2g[:], A3[:, :, 1])
                pf = apool.tile([128, NSLOT], dt.float32, name=f"pf_{dire}", tag="pf")
                nc.vector.tensor_copy(pf[:], P3[:, :, 0])

                # decode the winner's branch bits: bit0 (X level, weight 2048),
                # bit1 (F1 level, raw value 2 -> weight 1024), bit2 (F2 level,
                # raw value 4 -> weight 512)
                b0u = apool.tile([128, NSLOT], dt.uint16, name=f"b0u_{dire}", tag="b0u")
                nc.vector.tensor_scalar(b0u[:], A3u[:, :, 0], 1, None, op0=Alu.bitwise_and)
                b1u = apool.tile([128, NSLOT], dt.uint16, name=f"b1u_{dire}", tag="b1u")
                nc.vector.tensor_scalar(b1u[:], A3u[:, :, 0], 2, None, op0=Alu.bitwise_and)
                b2u = apool.tile([128, NSLOT], dt.uint16, name=f"b2u_{dire}", tag="b2u")
                nc.vector.tensor_scalar(b2u[:], A3u[:, :, 0], 4, None, op0=Alu.bitwise_and)
                b0f = apool.tile([128, NSLOT], dt.float32, name=f"b0f_{dire}", tag="b0f")
                nc.vector.tensor_copy(b0f[:], b0u[:])
                b1f = apool.tile([128, NSLOT], dt.float32, name=f"b1f_{dire}", tag="b1f")
                nc.vector.tensor_copy(b1f[:], b1u[:])
                b2f = apool.tile([128, NSLOT], dt.float32, name=f"b2f_{dire}", tag="b2f")
                nc.vector.tensor_copy(b2f[:], b2u[:])

                # absolute column index: m = p + 2048*b0 + 1024*(b1/2) + 512*(b2/4)
                mst = apool.tile([128, NSLOT], dt.float32, name=f"mst_{dire}", tag="mst")
                nc.vector.scalar_tensor_tensor(
                    mst[:], b0f[:], 2048.0, pf[:], op0=Alu.mult, op1=Alu.add
                )
                nc.vector.scalar_tensor_tensor(
                    mst[:], b1f[:], 512.0, mst[:], op0=Alu.mult, op1=Alu.add
                )
                nc.vector.scalar_tensor_tensor(
                    mst[:], b2f[:], 128.0, mst[:], op0=Alu.mult, op1=Alu.add
                )

                # ratio test: dist1 <= r^2 * dist2  <=>  v1 - r^2*v2 >= 1 - r^2
                acc1 = apool.tile([128, NT], dt.float32, name=f"acc1_{dire}", tag="acc1")
                nc.vector.scalar_tensor_tensor(
                    acc1[:], v2g[:], -RATIO2, v1g[:], op0=Alu.mult, op1=Alu.add
                )
                maskf = apool.tile([128, NT], dt.uint8, name=f"maskf_{dire}", tag="maskf")
                nc.vector.tensor_scalar(
                    maskf[:], acc1[:], 1.0 - RATIO2, None, op0=Alu.is_ge
                )
                if dire == 0:
                    sc = apool.tile([128, NT], dt.float32, name="sc")
                    nc.vector.tensor_scalar(
                        sc[:], v1g[:], 0.5, 0.5, op0=Alu.mult, op1=Alu.add
                    )
                    nc.vector.tensor_mul(scores0[:], sc[:], maskf[:])
                nc.vector.memset(m_dir[dire][:], -1.0)
                nc.vector.copy_predicated(m_dir[dire][:], maskf[:], mst[:])

            # ---- mutual check ----
            m1_flat_ap = m1_bounce[:].rearrange("(t r) -> r t", r=128)
            nc.sync.dma_start(m1_flat_ap, m_dir[1][:])
            m1_rep = apool.tile([128, M], dt.float32, name="m1_rep")
            nc.sync.dma_start(m1_rep[:1, :], m1_bounce[:][None, :])
            nc.gpsimd.partition_broadcast(m1_rep[:, :], m1_rep[:1, :])

            safe = apool.tile([128, NT], dt.float32, name="safe")
            nc.vector.tensor_scalar_max(safe[:], m_dir[0][:], 0.0)
            safe16 = apool.tile([128, NT], dt.uint16, name="safe16")
            nc.vector.tensor_copy(safe16[:], safe[:])
            gm = apool.tile([128, 16 * NT], dt.float32, name="gm")
            nc.gpsimd.indirect_copy(gm[:], m1_rep[:], safe16[:], True)
            gmp = apool.tile([128, 16 * NT], dt.float32, name="gmp")
            nc.vector.tensor_mul(gmp[:], gm[:], diag_f[:])
            loop = apool.tile([128, NT], dt.float32, name="loop")
            nc.vector.tensor_reduce(
                loop[:],
                gmp[:].rearrange("p (j u) -> p j u", u=16),
                axis=mybir.AxisListType.X,
                op=Alu.add,
            )

            g1 = apool.tile([128, NT], dt.uint8, name="g1")
            nc.vector.tensor_scalar(g1[:], m_dir[0][:], -0.5, None, op0=Alu.is_gt)
            g2 = apool.tile([128, NT], dt.uint8, name="g2")
            nc.vector.tensor_tensor(g2[:], indsn[:], loop[:], op=Alu.is_equal)
            okm = apool.tile([128, NT], dt.uint8, name="okm")
            nc.vector.tensor_mul(okm[:], g1[:], g2[:])

            mfin = apool.tile([128, NT], dt.float32, name="mfin")
            nc.vector.memset(mfin[:], -1.0)
            nc.vector.copy_predicated(mfin[:], okm[:], m_dir[0][:])
            mi32 = apool.tile([128, NT], dt.int32, name="mi32")
            nc.vector.tensor_copy(mi32[:], mfin[:])

            nc.sync.dma_start(matches_dram[:].rearrange("(t r) -> r t", r=128), mi32[:])
            nc.sync.dma_start(scores_dram[:].rearrange("(t r) -> r t", r=128), scores0[:])

    nc.compile()
    return nc


def _get_fast():
    if "fast" not in _CACHE:
        _CACHE["fast"] = _build_fast()
    return _CACHE["fast"]


def _get_slow():
    if "slow" not in _CACHE:
        _CACHE["slow"] = _build_slow()
    return _CACHE["slow"]


def _make_consts():
    if "consts" in _CACHE:
        return _CACHE["consts"]
    p = np.arange(128)
    c_indsn = (128 * np.arange(NT)[None, :] + p[:, None]).astype(np.float32)
    diag = (np.arange(16)[None, :] == (p % 16)[:, None])  # [128, 16]
    c_diagf = np.tile(diag, (1, NT)).astype(np.float32)
    consts = {"c_indsn": c_indsn, "c_diagf": c_diagf}
    _CACHE["consts"] = consts
    return consts


def _make_fast_in_maps(descriptors0, descriptors1):
    f8 = ml_dtypes.float8_e4m3
    in_maps = []
    for c in range(B):
        a = np.ascontiguousarray(
            (descriptors0[c] * FSCALE).reshape(2, 128, N)
        ).astype(f8)
        bb = np.ascontiguousarray(
            (descriptors1[c] * FSCALE).reshape(2, 128, M)
        ).astype(f8)
        in_maps.append({"d0f8": a, "d1f8": bb})
    return in_maps


def _make_slow_in_maps(descriptors0, descriptors1):
    consts = _make_consts()
    in_maps = []
    for c in range(B):
        a = np.ascontiguousarray(descriptors0[c].reshape(2, 128, N)).astype(
            ml_dtypes.bfloat16
        )
        bb = np.ascontiguousarray(descriptors1[c].reshape(2, 128, M)).astype(
            ml_dtypes.bfloat16
        )
        in_maps.append({"d0": a, "d1": bb, **consts})
    return in_maps


def kernel(descriptors0: np.ndarray, descriptors1: np.ndarray):
    from concourse.bass_utils import run_bass_kernel_spmd

    fast = _get_fast()
    fast_maps = _make_fast_in_maps(descriptors0, descriptors1)
    res = run_bass_kernel_spmd(fast, fast_maps, core_ids=list(range(B)))
    flags = [int(np.asarray(res.results[c]["flag"])[0]) for c in range(B)]
    if not any(flags):
        return (
            np.full((B, N), -1, dtype=np.int32),
            np.zeros((B, N), dtype=np.float32),
        )

    # potential matches exist (e.g. planted correspondences): run the exact
    # bf16 program with full argmax decode + mutual check
    slow = _get_slow()
    slow_maps = _make_slow_in_maps(descriptors0, descriptors1)
    res = run_bass_kernel_spmd(slow, slow_maps, core_ids=list(range(B)))
    matches = np.stack([np.asarray(res.results[c]["matches"]) for c in range(B)])
    scores = np.stack([np.asarray(res.results[c]["scores"]) for c in range(B)])
    return matches.astype(np.int32), scores.astype(np.float32)
